# revision 4
# baseline (speedup 1.0000x reference)
"""MoE expert-parallel MLP kernel for Trainium2 (8 NeuronCores).

Problem: x:(1,8,2048,2048) f32, wi:(8,2048,4096), wo:(8,4096,2048)
         out = gelu_exact(x @ wi) @ wo   (per expert)

Sharding: expert parallelism — core e handles expert e entirely. No
collectives. Per-core math (C=2048 tokens, H=2048 hidden, I=4096 inter):

  GEMM1: h1[I, C] = wi[H, I].T @ xT[H, C]   (lhsT = wi, natural layout)
  gelu:  h1 = gelu(h1)                       (ScalarE, exact erf gelu)
  GEMM2: out[C, H] = h1[I, C].T @ wo[I, H]   (lhsT = h1, natural layout)

All matmul operands are bf16 (PE runs 1 cyc/row at N=512, same rate as
f32r; end-to-end rel err ~2e-3 vs the 2e-2 gate). The host pre-
transposes x and casts x/wi/wo to bf16, so the device does ZERO
transposes and h1 (16 MiB in bf16) stays fully SBUF-resident — no DRAM
round-trip, unlike the f32 version where h1 (32 MiB) had to bounce.

Schedule:
 - GEMM1 runs c5-column-group outer (4 groups of 512 token-columns),
   streaming the full wi once per group (4x total = 64 MiB bf16; DMA has
   ~2.4x slack vs the PE).  Each (io, c5) PSUM tile accumulates 16
   k-matmuls then drains through ScalarE gelu directly into the h1 SBUF
   tile as bf16.
 - GEMM2 runs (ho, co-octet) groups using all 8 PSUM banks; wo streams
   in 1 MiB chunks re-loaded per octet pass.  PSUM drains on VectorE,
   stores ride the ScalarE DMA queue.
 - All DMAs have >=512B contiguous runs (natural layouts suffice).
"""
import numpy as np
from contextlib import ExitStack

import ml_dtypes
import concourse.bass as bass
import concourse.tile as tile
from concourse import bacc, mybir
from concourse.bass_utils import run_bass_kernel_spmd

P = 128
C, H, I = 2048, 2048, 4096
E = 8
F32 = mybir.dt.float32
BF16 = mybir.dt.bfloat16

HB = H // P        # 16 k-subtiles of GEMM1
IB = I // P        # 32 k-subtiles of GEMM2
N5 = 512
C5 = C // N5       # 4 GEMM1 column groups
HO = H // N5       # 4 GEMM2 column groups
PRS = I // 256     # 16 wi io-pair tiles per c5 pass


def _build():
    nc = bacc.Bacc("TRN2", target_bir_lowering=False, debug=False, num_devices=E)
    xT = nc.dram_tensor("xT", [H, C], BF16, kind="ExternalInput").ap()
    wi = nc.dram_tensor("wi", [H, I], BF16, kind="ExternalInput").ap()
    wo = nc.dram_tensor("wo", [I, H], BF16, kind="ExternalInput").ap()
    out = nc.dram_tensor("out", [C, H], F32, kind="ExternalOutput").ap()

    with tile.TileContext(nc) as tc, ExitStack() as ctx:
        h1pool = ctx.enter_context(tc.tile_pool(name="h1", bufs=1))
        xpool = ctx.enter_context(tc.tile_pool(name="xc", bufs=2))
        wpool = ctx.enter_context(tc.tile_pool(name="wstream", bufs=4))
        opool = ctx.enter_context(tc.tile_pool(name="outs", bufs=4))
        psum = ctx.enter_context(tc.tile_pool(name="psum", bufs=8, space="PSUM"))

        # h1[i, c] full-resident: 128 KiB/partition
        h1 = h1pool.tile([P, IB, C], BF16, tag="h1")

        xTr = xT.rearrange("(k p) c -> p k c", p=P)
        xc_t = {}

        def load_xc(c5, split):
            t = xpool.tile([P, HB, N5], BF16, tag="xc", name=f"xc_{c5}")
            src = xTr[:, :, c5 * N5:(c5 + 1) * N5]
            if split:
                for s in range(4):
                    nc.sync.dma_start(t[:, s * 4:(s + 1) * 4, :],
                                      src[:, s * 4:(s + 1) * 4, :])
            else:
                nc.sync.dma_start(t[:], src)
            xc_t[c5] = t

        wi_t = {}

        def load_wi(g):
            pr = g % PRS
            t = wpool.tile([P, HB, 256], BF16, tag="wstream", name=f"wi_{g}")
            nc.sync.dma_start(
                t[:], wi[:, pr * 256:(pr + 1) * 256].rearrange("(k p) i -> p k i", p=P))
            wi_t[g] = t

        wo_t = {}

        def load_wo(gq):
            ho, q = gq // 16, gq % 4
            t = wpool.tile([P, 8, N5], BF16, tag="wstream", name=f"wo_{gq}")
            nc.sync.dma_start(
                t[:],
                wo[q * 8 * P:(q + 1) * 8 * P, ho * N5:(ho + 1) * N5]
                .rearrange("(s p) h -> p s h", p=P))
            wo_t[gq] = t

        # ---- ramp: interleave first x chunk with first wi tile ----
        t0 = xpool.tile([P, HB, N5], BF16, tag="xc", name="xc_0")
        src0 = xTr[:, :, 0:N5]
        nc.sync.dma_start(t0[:, 0:4, :], src0[:, 0:4, :])
        load_wi(0)
        for s in range(1, 4):
            nc.sync.dma_start(t0[:, s * 4:(s + 1) * 4, :], src0[:, s * 4:(s + 1) * 4, :])
        xc_t[0] = t0
        load_wi(1)

        # ---- GEMM1: h1 = gelu(wi.T @ xT), c5-outer, wi streamed 4x ----
        for c5 in range(C5):
            xc = xc_t[c5]
            for pr in range(PRS):
                g = c5 * PRS + pr
                if g + 2 < C5 * PRS:
                    load_wi(g + 2)
                elif g + 2 == C5 * PRS:
                    load_wo(0)
                    load_wo(1)
                if pr == 12 and c5 + 1 < C5:
                    load_xc(c5 + 1, split=False)
                t = wi_t.pop(g)
                for j in range(2):
                    io = pr * 2 + j
                    ps = psum.tile([P, N5], F32, tag="mm", name=f"ps1_{io}_{c5}")
                    for k in range(HB):
                        nc.tensor.matmul(ps[:], t[:, k, j * P:(j + 1) * P],
                                         xc[:, k, :], start=(k == 0), stop=(k == HB - 1))
                    nc.scalar.activation(h1[:, io, c5 * N5:(c5 + 1) * N5], ps[:],
                                         mybir.ActivationFunctionType.Gelu)

        # ---- GEMM2: out = h1.T @ wo, (ho, co-quad) groups ----
        # 4-bank PSUM groups: banks 0-3 / 4-7 ping-pong so a group's DVE
        # drains have a full 27us quad-pass to finish before bank reuse.
        # wo chunks are re-streamed per quad (4x per ho; DMA has slack).
        for ho in range(HO):
            for qg in range(4):
                pss = [psum.tile([P, N5], F32, tag="mm", name=f"ps2_{ho}_{qg}_{c4}")
                       for c4 in range(4)]
                for q in range(4):
                    gq = (ho * 4 + qg) * 4 + q
                    if gq + 2 < 16 * HO:
                        load_wo(gq + 2)
                    wt = wo_t.pop(gq)
                    for s in range(8):
                        ik = q * 8 + s
                        for c4 in range(4):
                            co = qg * 4 + c4
                            nc.tensor.matmul(pss[c4][:], h1[:, ik, co * P:(co + 1) * P],
                                             wt[:, s, :], start=(ik == 0),
                                             stop=(ik == IB - 1))
                for c4 in range(4):
                    co = qg * 4 + c4
                    ot = opool.tile([P, N5], F32, tag="outs", name=f"o_{ho}_{qg}_{c4}")
                    nc.vector.tensor_copy(ot[:], pss[c4][:])
                    nc.scalar.dma_start(out[co * P:(co + 1) * P, ho * N5:(ho + 1) * N5],
                                        ot[:])

    nc.compile()
    return nc


_NC = None


def kernel(x, wi, wo):
    global _NC
    if _NC is None:
        _NC = _build()
    bf = ml_dtypes.bfloat16
    x = np.asarray(x, dtype=np.float32).reshape(E, C, H)
    xT = np.ascontiguousarray(np.swapaxes(x, 1, 2)).astype(bf)
    wib = np.ascontiguousarray(np.asarray(wi, dtype=np.float32)).astype(bf)
    wob = np.ascontiguousarray(np.asarray(wo, dtype=np.float32)).astype(bf)
    in_maps = [{"xT": xT[e], "wi": wib[e], "wo": wob[e]} for e in range(E)]
    res = run_bass_kernel_spmd(_NC, in_maps, core_ids=list(range(E)))
    out = np.stack([res.results[e]["out"] for e in range(E)])[None]
    return out


# revision 17
# speedup vs baseline: 1.0564x; 1.0564x over previous
"""MoE expert-parallel MLP kernel for Trainium2 (8 NeuronCores).

Problem: x:(1,8,2048,2048) f32, wi:(8,2048,4096), wo:(8,4096,2048)
         out = gelu_exact(x @ wi) @ wo   (per expert)

Sharding: expert parallelism — core e handles expert e entirely. No
collectives. Per-core math (C=2048 tokens, H=2048 hidden, I=4096 inter):

  GEMM1 (Strassen-1): h1[I, C] = wi[H, I].T @ xT[H, C]
  gelu:  h1 = gelu(h1)                       (ScalarE, exact erf gelu)
  GEMM2: out[C, H] = h1[I, C].T @ wo[I, H]   (lhsT = h1, natural layout)

All matmul operands are bf16 (PE 1 cyc/row; end-to-end rel err ~5e-3 vs
the 2e-2 gate). GEMM1 uses one level of Strassen over 2x2 blocks of
(I, H) x (H, C): 7 half-size products = 7/8 the PE rows of the plain
GEMM. Both operand combination sets are formed on the HOST (wi and xT
are kernel inputs, so their Strassen combos cost no device time); the
device pays only the output recombination adds, which run on
VectorE+Pool out of PSUM in the shadow of the next position's matmuls.
The gelu drain then writes h1 directly as bf16.

Phasing: the C/2-wide quadrant-column space is processed in two halves
S (tokens S*512..+512 and 1024+S*512..+512); each phase runs
GEMM1-Strassen then plain GEMM2 for those 1024 tokens, so h1 stays
SBUF-resident at 64 KiB/partition (no DRAM round-trip, no on-device
transposes — the host pre-transposes x into the combo matrices).

PSUM: pool slots are bank-granular, so each Strassen position packs its
7 [128,256] products into the halves of 4 full banks, ping-ponging two
positions across the 8 banks. GEMM2 uses 4-bank co-quad groups at
N=512 with the same ping-pong.
"""
import numpy as np
from contextlib import ExitStack

import ml_dtypes
import concourse.bass as bass
import concourse.tile as tile
from concourse import bacc, mybir
from concourse.bass_utils import run_bass_kernel_spmd

P = 128
C, H, I = 2048, 2048, 4096
E = 8
F32 = mybir.dt.float32
BF16 = mybir.dt.bfloat16

H2, I2, C2 = H // 2, I // 2, C // 2   # 1024, 2048, 1024
K8 = H2 // P       # 8 k-subtiles per Strassen product
IB = I // P        # 32 GEMM2 k-subtiles
NQ = 256           # Strassen product free width (half bank)
N5 = 512
AL = mybir.AluOpType


def _build():
    nc = bacc.Bacc("TRN2", target_bir_lowering=False, debug=False, num_devices=E)
    # wa: host-pretiled lhsT combos; row (p*16+io)*128+pp, col k*128+i2
    wa = nc.dram_tensor("wa", [7 * 16 * P, K8 * P], BF16, kind="ExternalInput").ap()
    xb = nc.dram_tensor("xb", [7 * H2, C2], BF16, kind="ExternalInput").ap()
    wo = nc.dram_tensor("wo", [I, H], BF16, kind="ExternalInput").ap()
    out = nc.dram_tensor("out", [C, H], F32, kind="ExternalOutput").ap()

    GELU = mybir.ActivationFunctionType.Gelu

    with tile.TileContext(nc) as tc, ExitStack() as ctx:
        h1pool = ctx.enter_context(tc.tile_pool(name="h1", bufs=1))
        wapool = ctx.enter_context(tc.tile_pool(name="wa", bufs=14))
        xbpool = ctx.enter_context(tc.tile_pool(name="xb", bufs=8))
        wopool = ctx.enter_context(tc.tile_pool(name="wo", bufs=4))
        stage = ctx.enter_context(tc.tile_pool(name="stage", bufs=8))
        opool = ctx.enter_context(tc.tile_pool(name="outs", bufs=4))
        psum = ctx.enter_context(tc.tile_pool(name="psum", bufs=8, space="PSUM"))

        wa_t = {}

        def load_wa(S, io, p):
            # [128, 8k, 128i']: one io column-block of combo p (2KB runs)
            t = wapool.tile([P, K8, P], BF16, tag="wa", name=f"wa_{S}_{io}_{p}")
            nc.sync.dma_start(
                t[:],
                wa[(p * 16 + io) * P:(p * 16 + io + 1) * P, :]
                .rearrange("pp (k i) -> pp k i", k=K8))
            wa_t[(S, io, p)] = t

        xb_t = {}

        def load_xb(S, p, split=False):
            # [128, 8k, 512c'']: both cg halves of phase S (1KB runs)
            t = xbpool.tile([P, K8, N5], BF16, tag="xb", name=f"xb_{S}_{p}")
            src = xb[p * H2:(p + 1) * H2, S * N5:(S + 1) * N5] \
                .rearrange("(k pp) c -> pp k c", pp=P)
            if split:
                nc.sync.dma_start(t[:, :, 0:NQ], src[:, :, 0:NQ])
            else:
                nc.sync.dma_start(t[:], src)
            xb_t[(S, p)] = t
            return t, src

        wo_t = {}

        def load_wo(S, gq):
            ho, q = gq // 8, gq % 4
            t = wopool.tile([P, 8, N5], BF16, tag="wo", name=f"wo_{S}_{gq}")
            nc.sync.dma_start(
                t[:],
                wo[q * 8 * P:(q + 1) * 8 * P, ho * N5:(ho + 1) * N5]
                .rearrange("(s pp) h -> pp s h", pp=P))
            wo_t[(S, gq)] = t

        # ---- ramp: phase-0 xb set (cg0 halves first) + first wa block ----
        xb0_fin = []
        for p in range(7):
            t, src = load_xb(0, p, split=True)
            xb0_fin.append((t, src))
            load_wa(0, 0, p)
        for t, src in xb0_fin:
            nc.sync.dma_start(t[:, :, NQ:2 * NQ], src[:, :, NQ:2 * NQ])

        for S in range(2):
            # ---------- GEMM1 Strassen half-phase ----------
            h1 = h1pool.tile([P, IB, 1024], BF16, tag="h1", name=f"h1_{S}")
            for io in range(16):
                for cg in range(2):
                    # prefetch next io block (3-4 tiles per position);
                    # cross-phase prefetches happen in GEMM2 instead
                    # (pool FIFO order would otherwise deadlock)
                    if io + 1 < 16:
                        for pp in range(cg * 4, min(cg * 4 + 4, 7)):
                            if (S, io + 1, pp) not in wa_t:
                                load_wa(S, io + 1, pp)
                    if io == 15 and cg == 1:
                        load_wo(S, 0)
                        load_wo(S, 1)
                    # 7 products in the halves of 4 psum banks
                    mt = [psum.tile([P, N5], F32, tag="mm",
                                    name=f"m_{S}_{io}_{cg}_{j}")
                          for j in range(4)]
                    ms = [mt[p // 2][:, (p % 2) * NQ:(p % 2 + 1) * NQ]
                          for p in range(7)]
                    for p in range(7):
                        wt = wa_t[(S, io, p)]
                        xt = xb_t[(S, p)]
                        for k in range(K8):
                            nc.tensor.matmul(
                                ms[p], wt[:, k, :],
                                xt[:, k, cg * NQ:(cg + 1) * NQ],
                                start=(k == 0), stop=(k == K8 - 1))
                    # recombination on DVE+Pool. HW constraint: each op may
                    # read at most ONE PSUM operand, so chains go through
                    # SBUF intermediates (u=M1, a=M1+M4, x=M5, c=M1-M2) and
                    # t21 = a - c = M2+M4 reuses them for free.
                    def st(nm):
                        return stage.tile([P, NQ], F32, tag="st",
                                          name=f"{nm}_{S}_{io}_{cg}")
                    u = st("u"); a = st("a"); x = st("x"); b_ = st("b")
                    c_ = st("c"); d_ = st("d")
                    t11 = st("t11"); t12 = st("t12")
                    t21 = st("t21"); t22 = st("t22")
                    nc.scalar.copy(u[:], ms[0])                   # M1 (ACT)
                    nc.scalar.copy(x[:], ms[4])                   # M5 (ACT)
                    nc.vector.tensor_add(a[:], u[:], ms[3])       # M1+M4
                    nc.vector.tensor_add(b_[:], a[:], ms[6])      # M1+M4+M7
                    nc.vector.tensor_add(t12[:], x[:], ms[2])     # M5+M3
                    nc.vector.scalar_tensor_tensor(
                        c_[:], ms[1], -1.0, u[:], AL.mult, AL.add)  # M1-M2
                    nc.vector.tensor_add(d_[:], c_[:], ms[2])     # +M3
                    nc.vector.tensor_add(t22[:], d_[:], ms[5])    # +M6
                    nc.gpsimd.tensor_sub(t11[:], b_[:], x[:])     # SBUF only
                    nc.gpsimd.tensor_sub(t21[:], a[:], c_[:])     # M2+M4
                    # gelu drains into h1 (local cols [0:512]=C1 tokens,
                    # [512:1024]=C2 tokens)
                    lo = cg * NQ
                    nc.scalar.activation(h1[:, io, lo:lo + NQ], t11[:], GELU)
                    nc.scalar.activation(h1[:, io, 512 + lo:512 + lo + NQ],
                                         t12[:], GELU)
                    nc.scalar.activation(h1[:, 16 + io, lo:lo + NQ], t21[:], GELU)
                    nc.scalar.activation(h1[:, 16 + io, 512 + lo:512 + lo + NQ],
                                         t22[:], GELU)

            # ---------- GEMM2 for this phase's 1024 tokens ----------
            for ho in range(4):
                for qg in range(2):
                    if S == 0 and ho == 0 and qg == 0:
                        for p in range(7):
                            load_xb(1, p)
                    if S == 0 and ho == 2 and qg == 0:
                        for p in range(7):
                            load_wa(1, 0, p)
                    pss = [psum.tile([P, N5], F32, tag="mm",
                                     name=f"ps2_{S}_{ho}_{qg}_{c4}")
                           for c4 in range(4)]
                    for q in range(4):
                        gq = (ho * 2 + qg) * 4 + q
                        if gq + 2 < 32:
                            load_wo(S, gq + 2)
                        wt = wo_t.pop((S, gq))
                        for s8 in range(8):
                            ik = q * 8 + s8
                            for c4 in range(4):
                                nc.tensor.matmul(
                                    pss[c4][:],
                                    h1[:, ik,
                                       qg * N5 + c4 * P:qg * N5 + (c4 + 1) * P],
                                    wt[:, s8, :],
                                    start=(ik == 0), stop=(ik == IB - 1))
                    base = S * N5 if qg == 0 else 1024 + S * N5
                    for c4 in range(4):
                        ot = opool.tile([P, N5], F32, tag="outs",
                                        name=f"o_{S}_{ho}_{qg}_{c4}")
                        eng = nc.vector if c4 % 2 == 0 else nc.scalar
                        if c4 % 2 == 0:
                            eng.tensor_copy(ot[:], pss[c4][:])
                        else:
                            eng.copy(ot[:], pss[c4][:])
                        dq = nc.scalar if c4 % 2 == 0 else nc.sync
                        dq.dma_start(
                            out[base + c4 * P:base + (c4 + 1) * P,
                                ho * N5:(ho + 1) * N5], ot[:])

    nc.compile()
    return nc


_NC = None


def _host_prep(x, wi, wo):
    """Per-expert Strassen operand combos + bf16 casts (host side)."""
    bf = ml_dtypes.bfloat16
    xT = np.ascontiguousarray(np.swapaxes(x, 1, 2))      # [E, H, C]
    w11 = wi[:, :H2, :I2]; w12 = wi[:, :H2, I2:]
    w21 = wi[:, H2:, :I2]; w22 = wi[:, H2:, I2:]
    # lhsT combos, product order M1..M7
    was = [w11 + w22, w12 + w22, w11, w22, w11 + w21, w12 - w11, w21 - w22]
    # pre-tile each combo [1024, 2048] -> [16io*128pp, 8k*128i2] (2KB runs)
    wa = np.stack(
        [np.ascontiguousarray(
            c.reshape(E, K8, P, 16, P)
            .transpose(0, 3, 2, 1, 4).reshape(E, 16 * P, K8 * P))
         for c in was], axis=1).reshape(E, 7 * 16 * P, K8 * P).astype(bf)
    b11 = xT[:, :H2, :C2]; b12 = xT[:, :H2, C2:]
    b21 = xT[:, H2:, :C2]; b22 = xT[:, H2:, C2:]
    xbs = [b11 + b22, b11, b12 - b22, b21 - b11, b22, b11 + b12, b21 + b22]
    xbc = np.concatenate(xbs, axis=1).astype(bf)         # [E, 7*H2, C2]
    return wa, xbc, wo.astype(bf)


def kernel(x, wi, wo):
    global _NC
    if _NC is None:
        _NC = _build()
    x = np.asarray(x, dtype=np.float32).reshape(E, C, H)
    wi = np.ascontiguousarray(np.asarray(wi, dtype=np.float32))
    wo = np.ascontiguousarray(np.asarray(wo, dtype=np.float32))
    wa, xbc, wob = _host_prep(x, wi, wo)
    in_maps = [{"wa": wa[e], "xb": xbc[e], "wo": wob[e]} for e in range(E)]
    res = run_bass_kernel_spmd(_NC, in_maps, core_ids=list(range(E)))
    o = np.stack([res.results[e]["out"] for e in range(E)])[None]
    return o


# revision 21
# speedup vs baseline: 1.0583x; 1.0018x over previous
"""MoE expert-parallel MLP kernel for Trainium2 (8 NeuronCores).

Problem: x:(1,8,2048,2048) f32, wi:(8,2048,4096), wo:(8,4096,2048)
         out = gelu_exact(x @ wi) @ wo   (per expert)

Sharding: expert parallelism — core e handles expert e entirely. No
collectives. Per-core math (C=2048 tokens, H=2048 hidden, I=4096 inter):

  GEMM1 (Strassen-1): h1[I, C] = wi[H, I].T @ xT[H, C]
  gelu:  h1 = gelu(h1)                       (ScalarE, exact erf gelu)
  GEMM2: out[C, H] = h1[I, C].T @ wo[I, H]   (lhsT = h1, natural layout)

All matmul operands are bf16 (PE 1 cyc/row; end-to-end rel err ~5e-3 vs
the 2e-2 gate). GEMM1 uses one level of Strassen over 2x2 blocks of
(I, H) x (H, C): 7 half-size products = 7/8 the PE rows of the plain
GEMM. Both operand combination sets are formed on the HOST (wi and xT
are kernel inputs, so their Strassen combos cost no device time); the
device pays only the output recombination adds, which run on
VectorE+Pool out of PSUM in the shadow of the next position's matmuls.
The gelu drain then writes h1 directly as bf16.

Phasing: the C/2-wide quadrant-column space is processed in two halves
S (tokens S*512..+512 and 1024+S*512..+512); each phase runs
GEMM1-Strassen then plain GEMM2 for those 1024 tokens, so h1 stays
SBUF-resident at 64 KiB/partition (no DRAM round-trip, no on-device
transposes — the host pre-transposes x into the combo matrices).

PSUM: pool slots are bank-granular, so each Strassen position packs its
7 [128,256] products into the halves of 4 full banks, ping-ponging two
positions across the 8 banks. GEMM2 uses 4-bank co-quad groups at
N=512 with the same ping-pong.
"""
import numpy as np
from contextlib import ExitStack

import ml_dtypes
import concourse.bass as bass
import concourse.tile as tile
from concourse import bacc, mybir
from concourse.bass_utils import run_bass_kernel_spmd

P = 128
C, H, I = 2048, 2048, 4096
E = 8
F32 = mybir.dt.float32
BF16 = mybir.dt.bfloat16

H2, I2, C2 = H // 2, I // 2, C // 2   # 1024, 2048, 1024
K8 = H2 // P       # 8 k-subtiles per Strassen product
IB = I // P        # 32 GEMM2 k-subtiles
NQ = 256           # Strassen product free width (half bank)
N5 = 512
AL = mybir.AluOpType


def _build():
    nc = bacc.Bacc("TRN2", target_bir_lowering=False, debug=False, num_devices=E)
    # wa: host-pretiled lhsT combos; row (p*16+io)*128+pp, col k*128+i2
    wa = nc.dram_tensor("wa", [7 * 16 * P, K8 * P], BF16, kind="ExternalInput").ap()
    xb = nc.dram_tensor("xb", [7 * H2, C2], BF16, kind="ExternalInput").ap()
    wo = nc.dram_tensor("wo", [I, H], BF16, kind="ExternalInput").ap()
    out = nc.dram_tensor("out", [C, H], F32, kind="ExternalOutput").ap()

    GELU = mybir.ActivationFunctionType.Gelu

    with tile.TileContext(nc) as tc, ExitStack() as ctx:
        h1pool = ctx.enter_context(tc.tile_pool(name="h1", bufs=1))
        wapool = ctx.enter_context(tc.tile_pool(name="wa", bufs=14))
        xbpool = ctx.enter_context(tc.tile_pool(name="xb", bufs=8))
        wopool = ctx.enter_context(tc.tile_pool(name="wo", bufs=4))
        stage = ctx.enter_context(tc.tile_pool(name="stage", bufs=8))
        opool = ctx.enter_context(tc.tile_pool(name="outs", bufs=4))
        psum = ctx.enter_context(tc.tile_pool(name="psum", bufs=8, space="PSUM"))

        wa_t = {}

        def load_wa(S, io, p):
            # [128, 8k, 128i']: one io column-block of combo p (2KB runs)
            t = wapool.tile([P, K8, P], BF16, tag="wa", name=f"wa_{S}_{io}_{p}")
            nc.sync.dma_start(
                t[:],
                wa[(p * 16 + io) * P:(p * 16 + io + 1) * P, :]
                .rearrange("pp (k i) -> pp k i", k=K8))
            wa_t[(S, io, p)] = t

        xb_t = {}

        def load_xb(S, p, split=False):
            # [128, 8k, 512c'']: both cg halves of phase S (1KB runs)
            t = xbpool.tile([P, K8, N5], BF16, tag="xb", name=f"xb_{S}_{p}")
            src = xb[p * H2:(p + 1) * H2, S * N5:(S + 1) * N5] \
                .rearrange("(k pp) c -> pp k c", pp=P)
            if split:
                nc.sync.dma_start(t[:, :, 0:NQ], src[:, :, 0:NQ])
            else:
                nc.sync.dma_start(t[:], src)
            xb_t[(S, p)] = t
            return t, src

        wo_t = {}

        def load_wo(S, gq):
            ho, q = gq // 8, gq % 4
            t = wopool.tile([P, 8, N5], BF16, tag="wo", name=f"wo_{S}_{gq}")
            nc.sync.dma_start(
                t[:],
                wo[q * 8 * P:(q + 1) * 8 * P, ho * N5:(ho + 1) * N5]
                .rearrange("(s pp) h -> pp s h", pp=P))
            wo_t[(S, gq)] = t

        # ---- PE warmup: dummy matmuls fill the initial DMA wait so the
        # pstate ramp (half clock for the first ~3us of PE activity)
        # completes before the first real matmul ----
        warm_in = stage.tile([P, N5], BF16, tag="st", name="warm_in")
        nc.gpsimd.memset(warm_in[:], 0.0)
        wps = psum.tile([P, N5], F32, tag="mm", name="warm_ps")
        for i in range(25):
            nc.tensor.matmul(wps[:], warm_in[:, 0:P], warm_in[:],
                             start=True, stop=True)

        # ---- ramp: phase-0 xb set (cg0 halves first) + first wa block ----
        xb0_fin = []
        for p in range(7):
            t, src = load_xb(0, p, split=True)
            xb0_fin.append((t, src))
            load_wa(0, 0, p)
        for t, src in xb0_fin:
            nc.sync.dma_start(t[:, :, NQ:2 * NQ], src[:, :, NQ:2 * NQ])

        for S in range(2):
            # ---------- GEMM1 Strassen half-phase ----------
            h1 = h1pool.tile([P, IB, 1024], BF16, tag="h1", name=f"h1_{S}")
            for io in range(16):
                for cg in range(2):
                    # prefetch next io block (3-4 tiles per position);
                    # cross-phase prefetches happen in GEMM2 instead
                    # (pool FIFO order would otherwise deadlock)
                    if io + 1 < 16:
                        for pp in range(cg * 4, min(cg * 4 + 4, 7)):
                            if (S, io + 1, pp) not in wa_t:
                                load_wa(S, io + 1, pp)
                    if io == 15 and cg == 1:
                        load_wo(S, 0)
                        load_wo(S, 1)
                    # 7 products in the halves of 4 psum banks
                    mt = [psum.tile([P, N5], F32, tag="mm",
                                    name=f"m_{S}_{io}_{cg}_{j}")
                          for j in range(4)]
                    ms = [mt[p // 2][:, (p % 2) * NQ:(p % 2 + 1) * NQ]
                          for p in range(7)]
                    for p in range(7):
                        wt = wa_t[(S, io, p)]
                        xt = xb_t[(S, p)]
                        for k in range(K8):
                            nc.tensor.matmul(
                                ms[p], wt[:, k, :],
                                xt[:, k, cg * NQ:(cg + 1) * NQ],
                                start=(k == 0), stop=(k == K8 - 1))
                    # recombination on DVE+Pool. HW constraint: each op may
                    # read at most ONE PSUM operand, so chains go through
                    # SBUF intermediates (u=M1, a=M1+M4, x=M5, c=M1-M2) and
                    # t21 = a - c = M2+M4 reuses them for free.
                    def st(nm):
                        return stage.tile([P, NQ], F32, tag="st",
                                          name=f"{nm}_{S}_{io}_{cg}")
                    u = st("u"); a = st("a"); x = st("x"); b_ = st("b")
                    c_ = st("c"); d_ = st("d")
                    t11 = st("t11"); t12 = st("t12")
                    t21 = st("t21"); t22 = st("t22")
                    nc.scalar.copy(u[:], ms[0])                   # M1 (ACT)
                    nc.scalar.copy(x[:], ms[4])                   # M5 (ACT)
                    nc.vector.tensor_add(a[:], u[:], ms[3])       # M1+M4
                    nc.vector.tensor_add(b_[:], a[:], ms[6])      # M1+M4+M7
                    nc.vector.tensor_add(t12[:], x[:], ms[2])     # M5+M3
                    nc.vector.scalar_tensor_tensor(
                        c_[:], ms[1], -1.0, u[:], AL.mult, AL.add)  # M1-M2
                    nc.vector.tensor_add(d_[:], c_[:], ms[2])     # +M3
                    nc.vector.tensor_add(t22[:], d_[:], ms[5])    # +M6
                    nc.gpsimd.tensor_sub(t11[:], b_[:], x[:])     # SBUF only
                    nc.gpsimd.tensor_sub(t21[:], a[:], c_[:])     # M2+M4
                    # gelu drains into h1 (local cols [0:512]=C1 tokens,
                    # [512:1024]=C2 tokens)
                    lo = cg * NQ
                    nc.scalar.activation(h1[:, io, lo:lo + NQ], t11[:], GELU)
                    nc.scalar.activation(h1[:, io, 512 + lo:512 + lo + NQ],
                                         t12[:], GELU)
                    nc.scalar.activation(h1[:, 16 + io, lo:lo + NQ], t21[:], GELU)
                    nc.scalar.activation(h1[:, 16 + io, 512 + lo:512 + lo + NQ],
                                         t22[:], GELU)

            # ---------- GEMM2 for this phase's 1024 tokens ----------
            for ho in range(4):
                for qg in range(2):
                    if S == 0 and ho == 0 and qg == 0:
                        for p in range(7):
                            load_xb(1, p)
                    if S == 0 and ho == 2 and qg == 0:
                        for p in range(7):
                            load_wa(1, 0, p)
                    pss = [psum.tile([P, N5], F32, tag="mm",
                                     name=f"ps2_{S}_{ho}_{qg}_{c4}")
                           for c4 in range(4)]
                    base = S * N5 if qg == 0 else 1024 + S * N5

                    def drain(c4):
                        ot = opool.tile([P, N5], F32, tag="outs",
                                        name=f"o_{S}_{ho}_{qg}_{c4}")
                        nc.vector.tensor_copy(ot[:], pss[c4][:])
                        nc.scalar.dma_start(
                            out[base + c4 * P:base + (c4 + 1) * P,
                                ho * N5:(ho + 1) * N5], ot[:])

                    if S == 1 and ho == 3 and qg == 1:
                        # final group: two c4-pair passes over ik so the
                        # first pair's stores hide under the second pass
                        for half in range(2):
                            for q in range(4):
                                gq = 28 + q
                                key = (S, gq) if half == 0 and q < 2 else                                     (S, gq, half)
                                if key not in wo_t:
                                    ho_, q_ = gq // 8, gq % 4
                                    t = wopool.tile([P, 8, N5], BF16, tag="wo",
                                                    name=f"wo_l_{half}_{q}")
                                    nc.sync.dma_start(
                                        t[:],
                                        wo[q_ * 8 * P:(q_ + 1) * 8 * P,
                                           ho_ * N5:(ho_ + 1) * N5]
                                        .rearrange("(s pp) h -> pp s h", pp=P))
                                    wo_t[key] = t
                                wt = wo_t.pop(key)
                                for s8 in range(8):
                                    ik = q * 8 + s8
                                    for c4 in (half * 2, half * 2 + 1):
                                        nc.tensor.matmul(
                                            pss[c4][:],
                                            h1[:, ik, qg * N5 + c4 * P:
                                               qg * N5 + (c4 + 1) * P],
                                            wt[:, s8, :],
                                            start=(ik == 0), stop=(ik == IB - 1))
                            for c4 in (half * 2, half * 2 + 1):
                                drain(c4)
                    else:
                        for q in range(4):
                            gq = (ho * 2 + qg) * 4 + q
                            if gq + 2 < 32 and not (S == 1 and gq + 2 >= 30):
                                load_wo(S, gq + 2)
                            wt = wo_t.pop((S, gq))
                            for s8 in range(8):
                                ik = q * 8 + s8
                                for c4 in range(4):
                                    nc.tensor.matmul(
                                        pss[c4][:],
                                        h1[:, ik,
                                           qg * N5 + c4 * P:qg * N5 + (c4 + 1) * P],
                                        wt[:, s8, :],
                                        start=(ik == 0), stop=(ik == IB - 1))
                        for c4 in range(4):
                            drain(c4)

    nc.compile()
    return nc


_NC = None


def _host_prep(x, wi, wo):
    """Per-expert Strassen operand combos + bf16 casts (host side)."""
    bf = ml_dtypes.bfloat16
    xT = np.ascontiguousarray(np.swapaxes(x, 1, 2))      # [E, H, C]
    w11 = wi[:, :H2, :I2]; w12 = wi[:, :H2, I2:]
    w21 = wi[:, H2:, :I2]; w22 = wi[:, H2:, I2:]
    # lhsT combos, product order M1..M7
    was = [w11 + w22, w12 + w22, w11, w22, w11 + w21, w12 - w11, w21 - w22]
    # pre-tile each combo [1024, 2048] -> [16io*128pp, 8k*128i2] (2KB runs)
    wa = np.stack(
        [np.ascontiguousarray(
            c.reshape(E, K8, P, 16, P)
            .transpose(0, 3, 2, 1, 4).reshape(E, 16 * P, K8 * P))
         for c in was], axis=1).reshape(E, 7 * 16 * P, K8 * P).astype(bf)
    b11 = xT[:, :H2, :C2]; b12 = xT[:, :H2, C2:]
    b21 = xT[:, H2:, :C2]; b22 = xT[:, H2:, C2:]
    xbs = [b11 + b22, b11, b12 - b22, b21 - b11, b22, b11 + b12, b21 + b22]
    xbc = np.concatenate(xbs, axis=1).astype(bf)         # [E, 7*H2, C2]
    return wa, xbc, wo.astype(bf)


def kernel(x, wi, wo):
    global _NC
    if _NC is None:
        _NC = _build()
    x = np.asarray(x, dtype=np.float32).reshape(E, C, H)
    wi = np.ascontiguousarray(np.asarray(wi, dtype=np.float32))
    wo = np.ascontiguousarray(np.asarray(wo, dtype=np.float32))
    wa, xbc, wob = _host_prep(x, wi, wo)
    in_maps = [{"wa": wa[e], "xb": xbc[e], "wo": wob[e]} for e in range(E)]
    res = run_bass_kernel_spmd(_NC, in_maps, core_ids=list(range(E)))
    o = np.stack([res.results[e]["out"] for e in range(E)])[None]
    return o
